# revision 25
# baseline (speedup 1.0000x reference)
"""Causal self-attention with RoPE on 8 TRN2 NeuronCores.

Sharding: tensor-parallel over heads (H=8 -> 1 head per core).
Each core computes, for its head h:
    q,k,v projections (bf16 matmuls, fp32 PSUM) -> RoPE (DVE stream_shuffle
    pair-swap + fp32 cos/sin tables)
    S^T chunks (128 keys x 512 queries) via K=64 row-paired concurrent
    matmuls into double-buffered 3-bank PSUM group tiles;
    P^T = exp(S^T/8) on ACT, one instruction per 3-bank group (bf16 out to
    SBUF-resident per-block P tiles), causal diag masking via affine_select
    y_u^T = [v | ones]^T-weighted PV matmuls (row 64 = softmax denominator),
    software-pipelined one i-block behind S/exp so PE never waits on exp
    out_u = y_u @ Wp_h^T on-device; host computes sum_h out_u_h / colsum_h.
"""
import sys

sys.path.insert(0, "/opt/trn_rl_repo")

import numpy as np
import ml_dtypes

import concourse.bass as bass
import concourse.mybir as mybir
import concourse.tile as tile
from concourse.bass_utils import run_bass_kernel_spmd

B, T, C, H = 1, 4096, 512, 8
HS = C // H  # 64
NCORES = 8
TB = 512           # t-block width for projections / i-block width for attention
NTB = T // TB      # 8
JC = 128           # j-chunk width
NJC = T // JC      # 32
G = 3              # j-chunks (PSUM banks) per exp group

_ctr = [0]


def _legalize_waits(nc):
    """This walrus build accepts at most one sem-wait command per hw
    instruction; move extra waits onto same-engine NoOps inserted before."""
    for f in nc.m.functions:
        for bb in f.blocks:
            insts = bb.instructions
            out = []
            for inst in insts:
                si = inst.sync_info
                if si is not None and len(si.on_wait) > 1:
                    waits = list(si.on_wait)
                    for w in waits[:-1]:
                        _ctr[0] += 1
                        nop = mybir.InstNoOp(name=f"I-waitsplit-{_ctr[0]}")
                        nop.engine = inst.engine
                        nop.sync_info = mybir.SyncInfo(on_wait=[w], on_update=[])
                        out.append(nop)
                    inst.sync_info = mybir.SyncInfo(
                        on_wait=[waits[-1]], on_update=list(si.on_update)
                    )
                out.append(inst)
            insts[:] = out
    return nc


# pair-swap within 32-partition quadrants: [1,0,3,2,...,31,30]
_SWAP_MASK = [i ^ 1 for i in range(32)]


def _build_nc(trace_scopes=False):
    nc = bass.Bass()
    f32 = mybir.dt.float32
    bf16 = mybir.dt.bfloat16

    xt_in = nc.declare_dram_parameter("xt", [C, T], bf16, isOutput=False)
    # wall = concat([wqk (C,128), wv (C,64)], axis=1) -> (C, 192)
    wall_in = nc.declare_dram_parameter("wall", [C, 192], bf16, isOutput=False)
    wp_in = nc.declare_dram_parameter("wp", [HS, C], bf16, isOutput=False)
    cc_in = nc.declare_dram_parameter("cc", [128, T], f32, isOutput=False)
    ss_in = nc.declare_dram_parameter("ss", [128, T], f32, isOutput=False)
    out_u = nc.declare_dram_parameter("out_u", [T, C], bf16, isOutput=True)
    cs_out = nc.declare_dram_parameter("cs", [1, T], f32, isOutput=True)

    Exp = mybir.ActivationFunctionType.Exp
    out_r = out_u.ap().rearrange("(b q p) c -> b p q c", p=128, q=4)

    with tile.TileContext(nc) as tc:
        with (
            tc.tile_pool(name="big", bufs=1) as big,
            tc.tile_pool(name="ropet", bufs=3) as ropet,
            tc.tile_pool(name="ptp", bufs=2) as ptp,
            tc.tile_pool(name="ytsb", bufs=2) as ytsb,
            tc.tile_pool(name="outp", bufs=2) as outp,
            tc.tile_pool(name="stp", bufs=2, space="PSUM") as stp,
            tc.tile_pool(name="pyt", bufs=1, space="PSUM") as pyt,
            tc.tile_pool(name="rot", bufs=1, space="PSUM") as rot,
        ):
            # ---- resident inputs (DMAs ordered so tb=0 deps land first) ----
            wall_sb = big.tile([128, 4, 192], bf16)
            nc.sync.dma_start(out=wall_sb,
                              in_=wall_in.ap().rearrange("(n p) m -> p n m", p=128))
            wqk_sb = wall_sb[:, :, 0:128]
            wv_sb = wall_sb[:, :, 128:192]
            wp_sb = big.tile([HS, C], bf16)
            nc.sync.dma_start(out=wp_sb, in_=wp_in.ap())
            xt_sb = big.tile([128, 4, T], bf16)
            _xt_r = xt_in.ap().rearrange("(n p) t -> p n t", p=128)
            cc_sb = big.tile([128, T], f32)
            ss_sb = big.tile([128, T], f32)
            # Input loads are issued just-in-time (512-col chunks) so the
            # startup-critical chunk-0/1 loads and the first krqr swap
            # copies are never queued behind megabytes of bulk traffic.
            _loaded = set()

            def load_chunk(k):
                if k in _loaded or k >= NTB:
                    return
                _loaded.add(k)
                t0 = k * TB
                nc.sync.dma_start(out=xt_sb[:, :, t0:t0 + TB],
                                  in_=_xt_r[:, :, t0:t0 + TB])
                nc.sync.dma_start(out=cc_sb[:, t0:t0 + TB],
                                  in_=cc_in.ap()[:, t0:t0 + TB])
                nc.sync.dma_start(out=ss_sb[:, t0:t0 + TB],
                                  in_=ss_in.ap()[:, t0:t0 + TB])

            load_chunk(0)
            load_chunk(1)

            qkr = big.tile([128, T], bf16)    # rows 0:64 = q_rot^T, 64:128 = k_rot^T
            krqr = big.tile([128, T], bf16)   # rows 0:64 = k_rot^T, 64:128 = q_rot^T
            v_ones = big.tile([128, NJC, HS + 1], bf16)
            nc.vector.memset(v_ones[:, :, HS], 1.0)
            cs_sb = big.tile([1, T], f32)

            yt_ps = pyt.tile([128, TB], f32)

            def proj_v(tb, scratch=False):
                """v projections for t-block tb: 4 j-chunks into one
                quarter-bank tile, single CAST.  scratch=True uses the yt
                bank (free before the first PV) so the prologue's two projs
                run on two banks concurrently."""
                tc0 = tb * TB
                if scratch:
                    for t4 in range(4):
                        p0 = tc0 + t4 * 128
                        for cn in range(4):
                            nc.tensor.matmul(yt_ps[:, t4 * HS:(t4 + 1) * HS],
                                             xt_sb[:, cn, p0:p0 + 128],
                                             wv_sb[:, cn, :],
                                             start=(cn == 0), stop=(cn == 3))
                        nc.vector.tensor_copy(
                            v_ones[:, 4 * tb + t4, 0:HS],
                            yt_ps[:, t4 * HS:(t4 + 1) * HS])
                    return
                v_ps = rot.tile([128, 4, HS], f32, tag="rv",
                                name=f"v_ps{tb}")
                for t4 in range(4):
                    p0 = tc0 + t4 * 128
                    for cn in range(4):
                        nc.tensor.matmul(v_ps[:, t4, :],
                                         xt_sb[:, cn, p0:p0 + 128],
                                         wv_sb[:, cn, :],
                                         start=(cn == 0), stop=(cn == 3))
                nc.vector.tensor_copy(v_ones[:, 4 * tb:4 * tb + 4, 0:HS], v_ps)

            def proj_qk(tb, scratch=False):
                """qk projection + rope for t-block tb."""
                tc0 = tb * TB
                if scratch:
                    qk_ps = yt_ps
                else:
                    qk_ps = rot.tile([128, TB], f32, tag="rv",
                                     name=f"qk_ps{tb}")
                for cn in range(4):
                    nc.tensor.matmul(qk_ps, wqk_sb[:, cn, :],
                                     xt_sb[:, cn, tc0:tc0 + TB],
                                     start=(cn == 0), stop=(cn == 3))
                # rope: qkr = qk*cc + pair_swap(qk)*ss   (ss sign-folded)
                qsw = ropet.tile([128, TB], f32, tag="rt")
                nc.vector.stream_shuffle(qsw, qk_ps, _SWAP_MASK)
                t1 = ropet.tile([128, TB], f32, tag="rt")
                nc.vector.tensor_mul(t1, qsw, ss_sb[:, tc0:tc0 + TB])
                t2 = ropet.tile([128, TB], f32, tag="rt")
                nc.vector.tensor_mul(t2, qk_ps, cc_sb[:, tc0:tc0 + TB])
                nc.vector.tensor_add(qkr[:, tc0:tc0 + TB], t2, t1)
                # swapped-halves duplicate for the row-paired S^T matmuls
                nc.sync.dma_start(out=krqr[0:64, tc0:tc0 + TB],
                                  in_=qkr[64:128, tc0:tc0 + TB])
                nc.sync.dma_start(out=krqr[64:128, tc0:tc0 + TB],
                                  in_=qkr[0:64, tc0:tc0 + TB])

            proj_v(1, scratch=True)
            proj_v(0)
            proj_qk(1, scratch=True)
            proj_qk(0)
            load_chunk(2)

            # Software pipeline over i-blocks: step ib runs S^T+exp for block
            # ib while PV matmuls for block ib-1 (P fully SBUF-resident)
            # interleave into the PE stream.
            pt_prev = None

            def emit_S(j, i0, st_g, off):
                v0 = max(0, j * JC - i0)
                if j % 2 == 0:
                    nc.tensor.matmul(
                        st_g[:, off, v0:TB],
                        krqr[0:64, j * JC:(j + 1) * JC],
                        qkr[0:64, i0 + v0:i0 + TB], tile_position=(0, 0))
                else:
                    nc.tensor.matmul(
                        st_g[:, off, v0:TB],
                        qkr[64:128, j * JC:(j + 1) * JC],
                        krqr[64:128, i0 + v0:i0 + TB], tile_position=(64, 0))

            def emit_exp(st_g, gsz, c0, pt_all, i0):
                nc.scalar.activation(pt_all[:, c0:c0 + gsz, :],
                                     st_g[:, 0:gsz, :], Exp, scale=0.125)
                for j in range(c0, c0 + gsz):
                    if j * JC + JC - 1 > i0:  # chunk touches the diagonal
                        b0 = j * JC - i0
                        b1 = min(TB, b0 + JC)
                        nc.gpsimd.affine_select(
                            out=pt_all[:, j, b0:b1], in_=pt_all[:, j, b0:b1],
                            compare_op=mybir.AluOpType.is_ge,
                            fill=0.0, base=0,
                            pattern=[[1, b1 - b0]], channel_multiplier=-1)

            def emit_pv(j, i0p, njp, pt):
                v0 = max(0, j * JC - i0p)
                nc.tensor.matmul(yt_ps[0:HS + 1, v0:TB],
                                 v_ones[:, j, :],
                                 pt[:, j, v0:TB],
                                 start=(j == 0), stop=(j == njp - 1),
                                 skip_group_check=True)

            def close_dve(ibp):
                """yt evac + denominator copy for block ibp; the c_proj
                matmuls are deferred into the next step's PE stream."""
                i0p = ibp * TB
                yt_sb = ytsb.tile([HS + 1, TB], bf16, tag="yts",
                                  name=f"yt_sb{ibp}")
                nc.vector.tensor_copy(yt_sb, yt_ps[0:HS + 1, :])
                nc.vector.tensor_copy(cs_sb[0:1, i0p:i0p + TB],
                                      yt_ps[HS:HS + 1, :])
                ot = outp.tile([128, 4, TB], bf16, tag="ot",
                               name=f"ot{ibp}")
                return {"ibp": ibp, "yt_sb": yt_sb, "ot": ot}

            def emit_op(pend, q):
                """One c_proj matmul+evac for the pending closed block.
                Scratch alternates rot bank / yt bank; all four ops are
                emitted before the first PV of the current block."""
                yt_sb, ot = pend["yt_sb"], pend["ot"]
                if q % 2 == 0:
                    op_ps = rot.tile([128, TB], f32, tag="rv",
                                     name=f"op_ps{pend['ibp']}_{q}")
                    nc.tensor.matmul(op_ps,
                                     yt_sb[0:HS, q * 128:(q + 1) * 128],
                                     wp_sb)
                    nc.vector.tensor_copy(ot[:, q, :], op_ps)
                else:
                    nc.tensor.matmul(yt_ps,
                                     yt_sb[0:HS, q * 128:(q + 1) * 128],
                                     wp_sb)
                    nc.vector.tensor_copy(ot[:, q, :], yt_ps)
                if q == 3:
                    nc.sync.dma_start(out=out_r[pend["ibp"]], in_=ot)

            # Block processing order: small block 0 last, so the trailing
            # PV-only pipeline stage drains 4 chunks instead of 32.
            order = [1, 2, 3, 4, 5, 6, 7, 0]
            for step in range(NTB + 1):
                cur = order[step] if step < NTB else None
                prv = order[step - 1] if step >= 1 else None
                njp = 4 * prv + 4 if prv is not None else 0
                i0p = prv * TB if prv is not None else 0
                load_chunk(step + 3)

                if cur is not None:
                    i0 = cur * TB
                    nj = 4 * cur + 4
                    ngroups = (nj + G - 1) // G
                    pt_all = ptp.tile([128, NJC, TB], bf16, tag="pt",
                                      name=f"pt_all{cur}")
                    st_tiles = {}
                    n_exp = 0
                    pv_i = 0
                    npairs = nj // 2
                    for m in range(npairs):
                        for j in (2 * m, 2 * m + 1):
                            gi = j // G
                            if gi not in st_tiles:
                                st_tiles[gi] = stp.tile(
                                    [128, G, TB], f32, tag="st",
                                    name=f"st_g{cur}_{gi}")
                            emit_S(j, i0, st_tiles[gi], j - gi * G)
                        if m == 0 and step + 2 < NTB:
                            proj_v(step + 2)
                            proj_qk(step + 2)
                        while (n_exp < ngroups
                               and min(nj, (n_exp + 1) * G) - 1 <= 2 * m + 1):
                            c0 = n_exp * G
                            gsz = min(nj, c0 + G) - c0
                            emit_exp(st_tiles[n_exp], gsz, c0, pt_all, i0)
                            n_exp += 1
                        if prv is not None:
                            tgt = min(njp, (m + 1) * njp // npairs)
                            while pv_i < tgt:
                                emit_pv(pv_i, i0p, njp, pt_prev)
                                pv_i += 1
                    while n_exp < ngroups:
                        c0 = n_exp * G
                        gsz = min(nj, c0 + G) - c0
                        emit_exp(st_tiles[n_exp], gsz, c0, pt_all, i0)
                        n_exp += 1
                    if prv is not None:
                        while pv_i < njp:
                            emit_pv(pv_i, i0p, njp, pt_prev)
                            pv_i += 1
                        pend = close_dve(prv)
                        for q in range(4):
                            emit_op(pend, q)
                    pt_prev = pt_all
                else:
                    for j in range(njp):
                        emit_pv(j, i0p, njp, pt_prev)
                    pend = close_dve(prv)
                    for q in range(4):
                        emit_op(pend, q)

            nc.sync.dma_start(out=cs_out.ap(), in_=cs_sb)

    _legalize_waits(nc)
    return nc


_cached = {}


def _get_nc():
    if "nc" not in _cached:
        _cached["nc"] = _build_nc()
    return _cached["nc"]


def _prep_inputs(x, rope, W_attn, W_proj):
    bf16 = ml_dtypes.bfloat16
    xt = np.ascontiguousarray(x[0].T).astype(bf16)          # (C, T)
    cos = np.asarray(rope[..., 0], dtype=np.float32)        # (T, HS//2)
    sin = np.asarray(rope[..., 1], dtype=np.float32)
    cc64 = np.repeat(cos.T, 2, axis=0)                      # (HS, T)
    ss64 = np.repeat(sin.T, 2, axis=0)
    ss64[0::2, :] *= -1.0                                   # sign folded: even rows -sin
    cc = np.ascontiguousarray(np.concatenate([cc64, cc64], axis=0))   # (128, T)
    ss = np.ascontiguousarray(np.concatenate([ss64, ss64], axis=0))

    Wa = np.asarray(W_attn, dtype=np.float32)
    Wp = np.asarray(W_proj, dtype=np.float32)

    in_maps = []
    for h in range(NCORES):
        Wq = Wa[h * HS:(h + 1) * HS]                        # (HS, C)
        Wk = Wa[C + h * HS:C + (h + 1) * HS]
        Wv = Wa[2 * C + h * HS:2 * C + (h + 1) * HS]
        wqk = np.concatenate([Wq.T, Wk.T], axis=1)                     # (C, 128)
        wv = Wv.T                                                      # (C, HS)
        wall = np.ascontiguousarray(
            np.concatenate([wqk, wv], axis=1)).astype(bf16)            # (C, 192)
        wp = np.ascontiguousarray(Wp[:, h * HS:(h + 1) * HS].T).astype(bf16)  # (HS, C)
        in_maps.append({
            "xt": xt, "wall": wall, "wp": wp, "cc": cc, "ss": ss,
        })
    return in_maps


def run_cores(x, rope, W_attn, W_proj, trace=False):
    """Returns (list of per-core result dicts, BassKernelResults)."""
    nc = _get_nc()
    in_maps = _prep_inputs(x, rope, W_attn, W_proj)
    res = run_bass_kernel_spmd(nc, in_maps, list(range(NCORES)), trace=trace)
    return res


def kernel(x, rope, mask, W_attn, W_proj):
    res = run_cores(x, rope, W_attn, W_proj, trace=False)
    out = np.zeros((T, C), dtype=np.float32)
    for h in range(NCORES):
        r = res.results[h]
        cs = np.asarray(r["cs"], dtype=np.float32).reshape(T, 1)
        out += np.asarray(r["out_u"], dtype=np.float32) / cs
    return out.reshape(B, T, C).astype(np.float32)


# revision 26
# speedup vs baseline: 1.0934x; 1.0934x over previous
"""Causal self-attention with RoPE on 8 TRN2 NeuronCores.

Sharding: tensor-parallel over heads (H=8 -> 1 head per core).
Each core computes, for its head h:
    q,k,v projections (bf16 matmuls, fp32 PSUM) -> RoPE (DVE stream_shuffle
    pair-swap + fp32 cos/sin tables)
    S^T chunks (128 keys x 512 queries) via K=64 row-paired concurrent
    matmuls into double-buffered 3-bank PSUM group tiles;
    P^T = exp(S^T/8) on ACT, one instruction per 3-bank group (bf16 out to
    SBUF-resident per-block P tiles), causal diag masking via affine_select
    y_u^T = [v | ones]^T-weighted PV matmuls (row 64 = softmax denominator),
    software-pipelined one i-block behind S/exp so PE never waits on exp
    out_u = y_u @ Wp_h^T on-device; host computes sum_h out_u_h / colsum_h.
"""
import sys

sys.path.insert(0, "/opt/trn_rl_repo")

import numpy as np
import ml_dtypes

import concourse.bass as bass
import concourse.mybir as mybir
import concourse.tile as tile
from concourse.bass_utils import run_bass_kernel_spmd

B, T, C, H = 1, 4096, 512, 8
HS = C // H  # 64
NCORES = 8
TB = 512           # t-block width for projections / i-block width for attention
NTB = T // TB      # 8
JC = 128           # j-chunk width
NJC = T // JC      # 32
G = 3              # j-chunks (PSUM banks) per exp group

_ctr = [0]


def _legalize_waits(nc):
    """This walrus build accepts at most one sem-wait command per hw
    instruction; move extra waits onto same-engine NoOps inserted before."""
    for f in nc.m.functions:
        for bb in f.blocks:
            insts = bb.instructions
            out = []
            for inst in insts:
                si = inst.sync_info
                if si is not None and len(si.on_wait) > 1:
                    waits = list(si.on_wait)
                    for w in waits[:-1]:
                        _ctr[0] += 1
                        nop = mybir.InstNoOp(name=f"I-waitsplit-{_ctr[0]}")
                        nop.engine = inst.engine
                        nop.sync_info = mybir.SyncInfo(on_wait=[w], on_update=[])
                        out.append(nop)
                    inst.sync_info = mybir.SyncInfo(
                        on_wait=[waits[-1]], on_update=list(si.on_update)
                    )
                out.append(inst)
            insts[:] = out
    return nc


# pair-swap within 32-partition quadrants: [1,0,3,2,...,31,30]
_SWAP_MASK = [i ^ 1 for i in range(32)]


def _build_nc(trace_scopes=False):
    nc = bass.Bass()
    f32 = mybir.dt.float32
    bf16 = mybir.dt.bfloat16

    xt_in = nc.declare_dram_parameter("xt", [C, T], bf16, isOutput=False)
    # wall = concat([wqk (C,128), wv (C,64)], axis=1) -> (C, 192)
    wall_in = nc.declare_dram_parameter("wall", [C, 192], bf16, isOutput=False)
    wp_in = nc.declare_dram_parameter("wp", [HS, C], bf16, isOutput=False)
    cc_in = nc.declare_dram_parameter("cc", [128, T], f32, isOutput=False)
    ss_in = nc.declare_dram_parameter("ss", [128, T], f32, isOutput=False)
    out_u = nc.declare_dram_parameter("out_u", [T, C], bf16, isOutput=True)
    cs_out = nc.declare_dram_parameter("cs", [1, T], f32, isOutput=True)

    Exp = mybir.ActivationFunctionType.Exp
    out_r = out_u.ap().rearrange("(b q p) c -> b p q c", p=128, q=4)

    with tile.TileContext(nc) as tc:
        with (
            tc.tile_pool(name="big", bufs=1) as big,
            tc.tile_pool(name="ropet", bufs=3) as ropet,
            tc.tile_pool(name="ptp", bufs=2) as ptp,
            tc.tile_pool(name="ytsb", bufs=2) as ytsb,
            tc.tile_pool(name="outp", bufs=2) as outp,
            tc.tile_pool(name="stp", bufs=2, space="PSUM") as stp,
            tc.tile_pool(name="pyt", bufs=1, space="PSUM") as pyt,
            tc.tile_pool(name="rot", bufs=1, space="PSUM") as rot,
        ):
            # ---- resident inputs (DMAs ordered so tb=0 deps land first) ----
            wall_sb = big.tile([128, 4, 192], bf16)
            nc.sync.dma_start(out=wall_sb,
                              in_=wall_in.ap().rearrange("(n p) m -> p n m", p=128))
            wqk_sb = wall_sb[:, :, 0:128]
            wv_sb = wall_sb[:, :, 128:192]
            wp_sb = big.tile([HS, C], bf16)
            nc.sync.dma_start(out=wp_sb, in_=wp_in.ap())
            xt_sb = big.tile([128, 4, T], bf16)
            _xt_r = xt_in.ap().rearrange("(n p) t -> p n t", p=128)
            cc_sb = big.tile([128, T], f32)
            ss_sb = big.tile([128, T], f32)
            # Input loads are issued just-in-time (512-col chunks) so the
            # startup-critical chunk-0/1 loads and the first krqr swap
            # copies are never queued behind megabytes of bulk traffic.
            _loaded = set()

            def load_chunk(k):
                if k in _loaded or k >= NTB:
                    return
                _loaded.add(k)
                t0 = k * TB
                nc.sync.dma_start(out=xt_sb[:, :, t0:t0 + TB],
                                  in_=_xt_r[:, :, t0:t0 + TB])
                nc.sync.dma_start(out=cc_sb[:, t0:t0 + TB],
                                  in_=cc_in.ap()[:, t0:t0 + TB])
                nc.sync.dma_start(out=ss_sb[:, t0:t0 + TB],
                                  in_=ss_in.ap()[:, t0:t0 + TB])

            load_chunk(0)
            load_chunk(1)

            qkr = big.tile([128, T], bf16)    # rows 0:64 = q_rot^T, 64:128 = k_rot^T
            krqr = big.tile([128, T], bf16)   # rows 0:64 = k_rot^T, 64:128 = q_rot^T
            v_ones = big.tile([128, NJC, HS + 1], bf16)
            nc.vector.memset(v_ones[:, :, HS], 1.0)
            cs_sb = big.tile([1, T], f32)

            yt_ps = pyt.tile([128, TB], f32)

            def proj_v(tb, scratch=False):
                """v projections for t-block tb: 4 j-chunks into one
                quarter-bank tile, single CAST.  scratch=True uses the yt
                bank (free before the first PV) so the prologue's two projs
                run on two banks concurrently."""
                tc0 = tb * TB
                if scratch:
                    for t4 in range(4):
                        p0 = tc0 + t4 * 128
                        for cn in range(4):
                            nc.tensor.matmul(yt_ps[:, t4 * HS:(t4 + 1) * HS],
                                             xt_sb[:, cn, p0:p0 + 128],
                                             wv_sb[:, cn, :],
                                             start=(cn == 0), stop=(cn == 3))
                        nc.vector.tensor_copy(
                            v_ones[:, 4 * tb + t4, 0:HS],
                            yt_ps[:, t4 * HS:(t4 + 1) * HS])
                    return
                v_ps = rot.tile([128, 4, HS], f32, tag="rv",
                                name=f"v_ps{tb}")
                for t4 in range(4):
                    p0 = tc0 + t4 * 128
                    for cn in range(4):
                        nc.tensor.matmul(v_ps[:, t4, :],
                                         xt_sb[:, cn, p0:p0 + 128],
                                         wv_sb[:, cn, :],
                                         start=(cn == 0), stop=(cn == 3))
                nc.vector.tensor_copy(v_ones[:, 4 * tb:4 * tb + 4, 0:HS], v_ps)

            def proj_qk(tb, scratch=False):
                """qk projection + rope for t-block tb."""
                tc0 = tb * TB
                if scratch:
                    qk_ps = yt_ps
                else:
                    qk_ps = rot.tile([128, TB], f32, tag="rv",
                                     name=f"qk_ps{tb}")
                for cn in range(4):
                    nc.tensor.matmul(qk_ps, wqk_sb[:, cn, :],
                                     xt_sb[:, cn, tc0:tc0 + TB],
                                     start=(cn == 0), stop=(cn == 3))
                # rope: qkr = qk*cc + pair_swap(qk)*ss   (ss sign-folded)
                qsw = ropet.tile([128, TB], f32, tag="rt")
                nc.vector.stream_shuffle(qsw, qk_ps, _SWAP_MASK)
                t1 = ropet.tile([128, TB], f32, tag="rt")
                nc.vector.tensor_mul(t1, qsw, ss_sb[:, tc0:tc0 + TB])
                t2 = ropet.tile([128, TB], f32, tag="rt")
                nc.vector.tensor_mul(t2, qk_ps, cc_sb[:, tc0:tc0 + TB])
                nc.vector.tensor_add(qkr[:, tc0:tc0 + TB], t2, t1)
                # swapped-halves duplicate for the row-paired S^T matmuls
                nc.sync.dma_start(out=krqr[0:64, tc0:tc0 + TB],
                                  in_=qkr[64:128, tc0:tc0 + TB])
                nc.sync.dma_start(out=krqr[64:128, tc0:tc0 + TB],
                                  in_=qkr[0:64, tc0:tc0 + TB])

            proj_v(0)
            proj_qk(0)
            load_chunk(2)
            proj_v(1)
            proj_qk(1)

            # Software pipeline over i-blocks: step ib runs S^T+exp for block
            # ib while PV matmuls for block ib-1 (P fully SBUF-resident)
            # interleave into the PE stream.
            pt_prev = None

            def emit_S(j, i0, st_g, off):
                v0 = max(0, j * JC - i0)
                if j % 2 == 0:
                    nc.tensor.matmul(
                        st_g[:, off, v0:TB],
                        krqr[0:64, j * JC:(j + 1) * JC],
                        qkr[0:64, i0 + v0:i0 + TB], tile_position=(0, 0))
                else:
                    nc.tensor.matmul(
                        st_g[:, off, v0:TB],
                        qkr[64:128, j * JC:(j + 1) * JC],
                        krqr[64:128, i0 + v0:i0 + TB], tile_position=(64, 0))

            def emit_exp(st_g, gsz, c0, pt_all, i0):
                nc.scalar.activation(pt_all[:, c0:c0 + gsz, :],
                                     st_g[:, 0:gsz, :], Exp, scale=0.125)
                for j in range(c0, c0 + gsz):
                    if j * JC + JC - 1 > i0:  # chunk touches the diagonal
                        b0 = j * JC - i0
                        b1 = min(TB, b0 + JC)
                        nc.gpsimd.affine_select(
                            out=pt_all[:, j, b0:b1], in_=pt_all[:, j, b0:b1],
                            compare_op=mybir.AluOpType.is_ge,
                            fill=0.0, base=0,
                            pattern=[[1, b1 - b0]], channel_multiplier=-1)

            def emit_pv(j, i0p, njp, pt):
                v0 = max(0, j * JC - i0p)
                nc.tensor.matmul(yt_ps[0:HS + 1, v0:TB],
                                 v_ones[:, j, :],
                                 pt[:, j, v0:TB],
                                 start=(j == 0), stop=(j == njp - 1),
                                 skip_group_check=True)

            def close_dve(ibp):
                """yt evac + denominator copy for block ibp; the c_proj
                matmuls are deferred into the next step's PE stream."""
                i0p = ibp * TB
                yt_sb = ytsb.tile([HS + 1, TB], bf16, tag="yts",
                                  name=f"yt_sb{ibp}")
                nc.vector.tensor_copy(yt_sb, yt_ps[0:HS + 1, :])
                nc.vector.tensor_copy(cs_sb[0:1, i0p:i0p + TB],
                                      yt_ps[HS:HS + 1, :])
                ot = outp.tile([128, 4, TB], bf16, tag="ot",
                               name=f"ot{ibp}")
                return {"ibp": ibp, "yt_sb": yt_sb, "ot": ot}

            def emit_op(pend, q):
                """One c_proj matmul+evac for the pending closed block.
                Scratch alternates rot bank / yt bank; all four ops are
                emitted before the first PV of the current block."""
                yt_sb, ot = pend["yt_sb"], pend["ot"]
                if q % 2 == 0:
                    op_ps = rot.tile([128, TB], f32, tag="rv",
                                     name=f"op_ps{pend['ibp']}_{q}")
                    nc.tensor.matmul(op_ps,
                                     yt_sb[0:HS, q * 128:(q + 1) * 128],
                                     wp_sb)
                    nc.vector.tensor_copy(ot[:, q, :], op_ps)
                else:
                    nc.tensor.matmul(yt_ps,
                                     yt_sb[0:HS, q * 128:(q + 1) * 128],
                                     wp_sb)
                    nc.vector.tensor_copy(ot[:, q, :], yt_ps)
                if q == 3:
                    nc.sync.dma_start(out=out_r[pend["ibp"]], in_=ot)

            # Block processing order: small block 0 last, so the trailing
            # PV-only pipeline stage drains 4 chunks instead of 32.
            order = [1, 2, 3, 4, 5, 6, 7, 0]
            for step in range(NTB + 1):
                cur = order[step] if step < NTB else None
                prv = order[step - 1] if step >= 1 else None
                njp = 4 * prv + 4 if prv is not None else 0
                i0p = prv * TB if prv is not None else 0
                load_chunk(step + 3)

                if cur is not None:
                    i0 = cur * TB
                    nj = 4 * cur + 4
                    ngroups = (nj + G - 1) // G
                    pt_all = ptp.tile([128, NJC, TB], bf16, tag="pt",
                                      name=f"pt_all{cur}")
                    st_tiles = {}
                    n_exp = 0
                    pv_i = 0
                    npairs = nj // 2
                    for m in range(npairs):
                        for j in (2 * m, 2 * m + 1):
                            gi = j // G
                            if gi not in st_tiles:
                                st_tiles[gi] = stp.tile(
                                    [128, G, TB], f32, tag="st",
                                    name=f"st_g{cur}_{gi}")
                            emit_S(j, i0, st_tiles[gi], j - gi * G)
                        if m == 0 and step + 2 < NTB:
                            proj_v(step + 2)
                            proj_qk(step + 2)
                        while (n_exp < ngroups
                               and min(nj, (n_exp + 1) * G) - 1 <= 2 * m + 1):
                            c0 = n_exp * G
                            gsz = min(nj, c0 + G) - c0
                            emit_exp(st_tiles[n_exp], gsz, c0, pt_all, i0)
                            n_exp += 1
                        if prv is not None:
                            tgt = min(njp, (m + 1) * njp // npairs)
                            while pv_i < tgt:
                                emit_pv(pv_i, i0p, njp, pt_prev)
                                pv_i += 1
                    while n_exp < ngroups:
                        c0 = n_exp * G
                        gsz = min(nj, c0 + G) - c0
                        emit_exp(st_tiles[n_exp], gsz, c0, pt_all, i0)
                        n_exp += 1
                    if prv is not None:
                        while pv_i < njp:
                            emit_pv(pv_i, i0p, njp, pt_prev)
                            pv_i += 1
                        pend = close_dve(prv)
                        for q in range(4):
                            emit_op(pend, q)
                    pt_prev = pt_all
                else:
                    for j in range(njp):
                        emit_pv(j, i0p, njp, pt_prev)
                    pend = close_dve(prv)
                    for q in range(4):
                        emit_op(pend, q)

            nc.sync.dma_start(out=cs_out.ap(), in_=cs_sb)

    _legalize_waits(nc)
    return nc


_cached = {}


def _get_nc():
    if "nc" not in _cached:
        _cached["nc"] = _build_nc()
    return _cached["nc"]


def _prep_inputs(x, rope, W_attn, W_proj):
    bf16 = ml_dtypes.bfloat16
    xt = np.ascontiguousarray(x[0].T).astype(bf16)          # (C, T)
    cos = np.asarray(rope[..., 0], dtype=np.float32)        # (T, HS//2)
    sin = np.asarray(rope[..., 1], dtype=np.float32)
    cc64 = np.repeat(cos.T, 2, axis=0)                      # (HS, T)
    ss64 = np.repeat(sin.T, 2, axis=0)
    ss64[0::2, :] *= -1.0                                   # sign folded: even rows -sin
    cc = np.ascontiguousarray(np.concatenate([cc64, cc64], axis=0))   # (128, T)
    ss = np.ascontiguousarray(np.concatenate([ss64, ss64], axis=0))

    Wa = np.asarray(W_attn, dtype=np.float32)
    Wp = np.asarray(W_proj, dtype=np.float32)

    in_maps = []
    for h in range(NCORES):
        Wq = Wa[h * HS:(h + 1) * HS]                        # (HS, C)
        Wk = Wa[C + h * HS:C + (h + 1) * HS]
        Wv = Wa[2 * C + h * HS:2 * C + (h + 1) * HS]
        wqk = np.concatenate([Wq.T, Wk.T], axis=1)                     # (C, 128)
        wv = Wv.T                                                      # (C, HS)
        wall = np.ascontiguousarray(
            np.concatenate([wqk, wv], axis=1)).astype(bf16)            # (C, 192)
        wp = np.ascontiguousarray(Wp[:, h * HS:(h + 1) * HS].T).astype(bf16)  # (HS, C)
        in_maps.append({
            "xt": xt, "wall": wall, "wp": wp, "cc": cc, "ss": ss,
        })
    return in_maps


def run_cores(x, rope, W_attn, W_proj, trace=False):
    """Returns (list of per-core result dicts, BassKernelResults)."""
    nc = _get_nc()
    in_maps = _prep_inputs(x, rope, W_attn, W_proj)
    res = run_bass_kernel_spmd(nc, in_maps, list(range(NCORES)), trace=trace)
    return res


def kernel(x, rope, mask, W_attn, W_proj):
    res = run_cores(x, rope, W_attn, W_proj, trace=False)
    out = np.zeros((T, C), dtype=np.float32)
    for h in range(NCORES):
        r = res.results[h]
        cs = np.asarray(r["cs"], dtype=np.float32).reshape(T, 1)
        out += np.asarray(r["out_u"], dtype=np.float32) / cs
    return out.reshape(B, T, C).astype(np.float32)


# revision 28
# speedup vs baseline: 1.1556x; 1.0569x over previous
"""Causal self-attention with RoPE on 8 TRN2 NeuronCores.

Sharding: tensor-parallel over heads (H=8 -> 1 head per core).
Each core computes, for its head h:
    q,k,v projections (bf16 matmuls, fp32 PSUM) -> RoPE (DVE stream_shuffle
    pair-swap + fp32 cos/sin tables)
    S^T chunks (128 keys x 512 queries) via K=64 row-paired concurrent
    matmuls into double-buffered 3-bank PSUM group tiles;
    P^T = exp(S^T/8) on ACT, one instruction per 3-bank group (bf16 out to
    SBUF-resident per-block P tiles), causal diag masking via affine_select
    y_u^T = [v | ones]^T-weighted PV matmuls (row 64 = softmax denominator),
    software-pipelined one i-block behind S/exp so PE never waits on exp
    out_u = y_u @ Wp_h^T on-device; host computes sum_h out_u_h / colsum_h.
"""
import sys

sys.path.insert(0, "/opt/trn_rl_repo")

import numpy as np
import ml_dtypes

import concourse.bass as bass
import concourse.mybir as mybir
import concourse.tile as tile
from concourse.bass_utils import run_bass_kernel_spmd

B, T, C, H = 1, 4096, 512, 8
HS = C // H  # 64
NCORES = 8
TB = 512           # t-block width for projections / i-block width for attention
NTB = T // TB      # 8
JC = 128           # j-chunk width
NJC = T // JC      # 32
G = 3              # j-chunks (PSUM banks) per exp group

_ctr = [0]


def _legalize_waits(nc):
    """This walrus build accepts at most one sem-wait command per hw
    instruction; move extra waits onto same-engine NoOps inserted before."""
    for f in nc.m.functions:
        for bb in f.blocks:
            insts = bb.instructions
            out = []
            for inst in insts:
                si = inst.sync_info
                if si is not None and len(si.on_wait) > 1:
                    waits = list(si.on_wait)
                    for w in waits[:-1]:
                        _ctr[0] += 1
                        nop = mybir.InstNoOp(name=f"I-waitsplit-{_ctr[0]}")
                        nop.engine = inst.engine
                        nop.sync_info = mybir.SyncInfo(on_wait=[w], on_update=[])
                        out.append(nop)
                    inst.sync_info = mybir.SyncInfo(
                        on_wait=[waits[-1]], on_update=list(si.on_update)
                    )
                out.append(inst)
            insts[:] = out
    return nc


# pair-swap within 32-partition quadrants: [1,0,3,2,...,31,30]
_SWAP_MASK = [i ^ 1 for i in range(32)]


def _build_nc(trace_scopes=False):
    nc = bass.Bass()
    f32 = mybir.dt.float32
    bf16 = mybir.dt.bfloat16

    xt_in = nc.declare_dram_parameter("xt", [C, T], bf16, isOutput=False)
    # wall = concat([wqk (C,128), wv (C,64)], axis=1) -> (C, 192)
    wall_in = nc.declare_dram_parameter("wall", [C, 192], bf16, isOutput=False)
    wp_in = nc.declare_dram_parameter("wp", [HS, C], bf16, isOutput=False)
    cc_in = nc.declare_dram_parameter("cc", [128, T], f32, isOutput=False)
    ss_in = nc.declare_dram_parameter("ss", [128, T], f32, isOutput=False)
    out_u = nc.declare_dram_parameter("out_u", [T, C], bf16, isOutput=True)
    cs_out = nc.declare_dram_parameter("cs", [1, T], f32, isOutput=True)

    Exp = mybir.ActivationFunctionType.Exp
    out_r = out_u.ap().rearrange("(b q p) c -> b p q c", p=128, q=4)

    with tile.TileContext(nc) as tc:
        with (
            tc.tile_pool(name="big", bufs=1) as big,
            tc.tile_pool(name="ropet", bufs=3) as ropet,
            tc.tile_pool(name="ptp", bufs=2) as ptp,
            tc.tile_pool(name="ytsb", bufs=2) as ytsb,
            tc.tile_pool(name="outp", bufs=2) as outp,
            tc.tile_pool(name="stp", bufs=2, space="PSUM") as stp,
            tc.tile_pool(name="pyt", bufs=1, space="PSUM") as pyt,
            tc.tile_pool(name="rot", bufs=1, space="PSUM") as rot,
        ):
            # ---- resident inputs (DMAs ordered so tb=0 deps land first) ----
            wall_sb = big.tile([128, 4, 192], bf16)
            nc.sync.dma_start(out=wall_sb,
                              in_=wall_in.ap().rearrange("(n p) m -> p n m", p=128))
            wqk_sb = wall_sb[:, :, 0:128]
            wv_sb = wall_sb[:, :, 128:192]
            wp_sb = big.tile([HS, C], bf16)
            xt_sb = big.tile([128, 4, T], bf16)
            _xt_r = xt_in.ap().rearrange("(n p) t -> p n t", p=128)
            cc_sb = big.tile([128, T], f32)
            ss_sb = big.tile([128, T], f32)
            # Input loads are issued just-in-time (512-col chunks) so the
            # startup-critical chunk-0/1 loads and the first krqr swap
            # copies are never queued behind megabytes of bulk traffic.
            _loaded = set()

            def load_chunk(k):
                if k in _loaded or k >= NTB:
                    return
                _loaded.add(k)
                t0 = k * TB
                nc.sync.dma_start(out=xt_sb[:, :, t0:t0 + TB],
                                  in_=_xt_r[:, :, t0:t0 + TB])
                nc.sync.dma_start(out=cc_sb[:, t0:t0 + TB],
                                  in_=cc_in.ap()[:, t0:t0 + TB])
                nc.sync.dma_start(out=ss_sb[:, t0:t0 + TB],
                                  in_=ss_in.ap()[:, t0:t0 + TB])

            load_chunk(0)
            load_chunk(1)
            nc.sync.dma_start(out=wp_sb, in_=wp_in.ap())

            qkr = big.tile([128, T], bf16)    # rows 0:64 = q_rot^T, 64:128 = k_rot^T
            krqr = big.tile([128, T], bf16)   # rows 0:64 = k_rot^T, 64:128 = q_rot^T
            v_ones = big.tile([128, NJC, HS + 1], bf16)
            nc.vector.memset(v_ones[:, :, HS], 1.0)
            cs_sb = big.tile([1, T], f32)

            yt_ps = pyt.tile([128, TB], f32)

            def proj_v(tb, scratch=False):
                """v projections for t-block tb: 4 j-chunks into one
                quarter-bank tile, single CAST.  scratch=True uses the yt
                bank (free before the first PV) so the prologue's two projs
                run on two banks concurrently."""
                tc0 = tb * TB
                if scratch:
                    for t4 in range(4):
                        p0 = tc0 + t4 * 128
                        for cn in range(4):
                            nc.tensor.matmul(yt_ps[:, t4 * HS:(t4 + 1) * HS],
                                             xt_sb[:, cn, p0:p0 + 128],
                                             wv_sb[:, cn, :],
                                             start=(cn == 0), stop=(cn == 3))
                        nc.vector.tensor_copy(
                            v_ones[:, 4 * tb + t4, 0:HS],
                            yt_ps[:, t4 * HS:(t4 + 1) * HS])
                    return
                v_ps = rot.tile([128, 4, HS], f32, tag="rv",
                                name=f"v_ps{tb}")
                for t4 in range(4):
                    p0 = tc0 + t4 * 128
                    for cn in range(4):
                        nc.tensor.matmul(v_ps[:, t4, :],
                                         xt_sb[:, cn, p0:p0 + 128],
                                         wv_sb[:, cn, :],
                                         start=(cn == 0), stop=(cn == 3))
                nc.vector.tensor_copy(v_ones[:, 4 * tb:4 * tb + 4, 0:HS], v_ps)

            def proj_qk(tb, scratch=False):
                """qk projection + rope for t-block tb."""
                tc0 = tb * TB
                if scratch:
                    qk_ps = yt_ps
                else:
                    qk_ps = rot.tile([128, TB], f32, tag="rv",
                                     name=f"qk_ps{tb}")
                for cn in range(4):
                    nc.tensor.matmul(qk_ps, wqk_sb[:, cn, :],
                                     xt_sb[:, cn, tc0:tc0 + TB],
                                     start=(cn == 0), stop=(cn == 3))
                # rope: qkr = qk*cc + pair_swap(qk)*ss   (ss sign-folded)
                qsw = ropet.tile([128, TB], f32, tag="rt")
                nc.vector.stream_shuffle(qsw, qk_ps, _SWAP_MASK)
                t1 = ropet.tile([128, TB], f32, tag="rt")
                nc.vector.tensor_mul(t1, qsw, ss_sb[:, tc0:tc0 + TB])
                t2 = ropet.tile([128, TB], f32, tag="rt")
                nc.vector.tensor_mul(t2, qk_ps, cc_sb[:, tc0:tc0 + TB])
                nc.vector.tensor_add(qkr[:, tc0:tc0 + TB], t2, t1)
                # swapped-halves duplicate for the row-paired S^T matmuls
                nc.sync.dma_start(out=krqr[0:64, tc0:tc0 + TB],
                                  in_=qkr[64:128, tc0:tc0 + TB])
                nc.sync.dma_start(out=krqr[64:128, tc0:tc0 + TB],
                                  in_=qkr[0:64, tc0:tc0 + TB])

            proj_v(0)
            proj_qk(0)
            load_chunk(2)
            proj_v(1)
            proj_qk(1)

            # Software pipeline over i-blocks: step ib runs S^T+exp for block
            # ib while PV matmuls for block ib-1 (P fully SBUF-resident)
            # interleave into the PE stream.
            pt_prev = None

            def emit_S(j, i0, st_g, off):
                v0 = max(0, j * JC - i0)
                if j % 2 == 0:
                    nc.tensor.matmul(
                        st_g[:, off, v0:TB],
                        krqr[0:64, j * JC:(j + 1) * JC],
                        qkr[0:64, i0 + v0:i0 + TB], tile_position=(0, 0))
                else:
                    nc.tensor.matmul(
                        st_g[:, off, v0:TB],
                        qkr[64:128, j * JC:(j + 1) * JC],
                        krqr[64:128, i0 + v0:i0 + TB], tile_position=(64, 0))

            def emit_exp(st_g, gsz, c0, pt_all, i0):
                nc.scalar.activation(pt_all[:, c0:c0 + gsz, :],
                                     st_g[:, 0:gsz, :], Exp, scale=0.125)
                for j in range(c0, c0 + gsz):
                    if j * JC + JC - 1 > i0:  # chunk touches the diagonal
                        b0 = j * JC - i0
                        b1 = min(TB, b0 + JC)
                        nc.gpsimd.affine_select(
                            out=pt_all[:, j, b0:b1], in_=pt_all[:, j, b0:b1],
                            compare_op=mybir.AluOpType.is_ge,
                            fill=0.0, base=0,
                            pattern=[[1, b1 - b0]], channel_multiplier=-1)

            def emit_pv(j, i0p, njp, pt):
                v0 = max(0, j * JC - i0p)
                nc.tensor.matmul(yt_ps[0:HS + 1, v0:TB],
                                 v_ones[:, j, :],
                                 pt[:, j, v0:TB],
                                 start=(j == 0), stop=(j == njp - 1),
                                 skip_group_check=True)

            def close_dve(ibp):
                """yt evac + denominator copy for block ibp; the c_proj
                matmuls are deferred into the next step's PE stream."""
                i0p = ibp * TB
                yt_sb = ytsb.tile([HS + 1, TB], bf16, tag="yts",
                                  name=f"yt_sb{ibp}")
                nc.any.tensor_copy(yt_sb, yt_ps[0:HS + 1, :])
                nc.any.tensor_copy(cs_sb[0:1, i0p:i0p + TB],
                                   yt_ps[HS:HS + 1, :])
                ot = outp.tile([128, 4, TB], bf16, tag="ot",
                               name=f"ot{ibp}")
                return {"ibp": ibp, "yt_sb": yt_sb, "ot": ot}

            def emit_op(pend, q):
                """One c_proj matmul+evac for the pending closed block.
                Scratch alternates rot bank / yt bank; all four ops are
                emitted before the first PV of the current block."""
                yt_sb, ot = pend["yt_sb"], pend["ot"]
                if q % 2 == 0:
                    op_ps = rot.tile([128, TB], f32, tag="rv",
                                     name=f"op_ps{pend['ibp']}_{q}")
                    nc.tensor.matmul(op_ps,
                                     yt_sb[0:HS, q * 128:(q + 1) * 128],
                                     wp_sb)
                    nc.any.tensor_copy(ot[:, q, :], op_ps)
                else:
                    nc.tensor.matmul(yt_ps,
                                     yt_sb[0:HS, q * 128:(q + 1) * 128],
                                     wp_sb)
                    nc.any.tensor_copy(ot[:, q, :], yt_ps)
                if q == 3:
                    nc.sync.dma_start(out=out_r[pend["ibp"]], in_=ot)

            # Block processing order: small block 0 last, so the trailing
            # PV-only pipeline stage drains 4 chunks instead of 32.
            order = [1, 2, 3, 4, 5, 6, 7, 0]
            for step in range(NTB + 1):
                cur = order[step] if step < NTB else None
                prv = order[step - 1] if step >= 1 else None
                njp = 4 * prv + 4 if prv is not None else 0
                i0p = prv * TB if prv is not None else 0
                load_chunk(step + 3)

                if cur is not None:
                    i0 = cur * TB
                    nj = 4 * cur + 4
                    ngroups = (nj + G - 1) // G
                    pt_all = ptp.tile([128, NJC, TB], bf16, tag="pt",
                                      name=f"pt_all{cur}")
                    st_tiles = {}
                    n_exp = 0
                    pv_i = 0
                    npairs = nj // 2
                    for m in range(npairs):
                        for j in (2 * m, 2 * m + 1):
                            gi = j // G
                            if gi not in st_tiles:
                                st_tiles[gi] = stp.tile(
                                    [128, G, TB], f32, tag="st",
                                    name=f"st_g{cur}_{gi}")
                            emit_S(j, i0, st_tiles[gi], j - gi * G)
                        if m == 0 and step + 2 < NTB:
                            proj_v(step + 2)
                            proj_qk(step + 2)
                        while (n_exp < ngroups
                               and min(nj, (n_exp + 1) * G) - 1 <= 2 * m + 1):
                            c0 = n_exp * G
                            gsz = min(nj, c0 + G) - c0
                            emit_exp(st_tiles[n_exp], gsz, c0, pt_all, i0)
                            n_exp += 1
                        if prv is not None:
                            tgt = min(njp, (m + 1) * njp // npairs)
                            while pv_i < tgt:
                                emit_pv(pv_i, i0p, njp, pt_prev)
                                pv_i += 1
                    while n_exp < ngroups:
                        c0 = n_exp * G
                        gsz = min(nj, c0 + G) - c0
                        emit_exp(st_tiles[n_exp], gsz, c0, pt_all, i0)
                        n_exp += 1
                    if prv is not None:
                        while pv_i < njp:
                            emit_pv(pv_i, i0p, njp, pt_prev)
                            pv_i += 1
                        pend = close_dve(prv)
                        for q in range(4):
                            emit_op(pend, q)
                    pt_prev = pt_all
                else:
                    for j in range(njp):
                        emit_pv(j, i0p, njp, pt_prev)
                    pend = close_dve(prv)
                    for q in range(4):
                        emit_op(pend, q)

            nc.sync.dma_start(out=cs_out.ap(), in_=cs_sb)

    _legalize_waits(nc)
    return nc


_cached = {}


def _get_nc():
    if "nc" not in _cached:
        _cached["nc"] = _build_nc()
    return _cached["nc"]


def _prep_inputs(x, rope, W_attn, W_proj):
    bf16 = ml_dtypes.bfloat16
    xt = np.ascontiguousarray(x[0].T).astype(bf16)          # (C, T)
    cos = np.asarray(rope[..., 0], dtype=np.float32)        # (T, HS//2)
    sin = np.asarray(rope[..., 1], dtype=np.float32)
    cc64 = np.repeat(cos.T, 2, axis=0)                      # (HS, T)
    ss64 = np.repeat(sin.T, 2, axis=0)
    ss64[0::2, :] *= -1.0                                   # sign folded: even rows -sin
    cc = np.ascontiguousarray(np.concatenate([cc64, cc64], axis=0))   # (128, T)
    ss = np.ascontiguousarray(np.concatenate([ss64, ss64], axis=0))

    Wa = np.asarray(W_attn, dtype=np.float32)
    Wp = np.asarray(W_proj, dtype=np.float32)

    in_maps = []
    for h in range(NCORES):
        Wq = Wa[h * HS:(h + 1) * HS]                        # (HS, C)
        Wk = Wa[C + h * HS:C + (h + 1) * HS]
        Wv = Wa[2 * C + h * HS:2 * C + (h + 1) * HS]
        wqk = np.concatenate([Wq.T, Wk.T], axis=1)                     # (C, 128)
        wv = Wv.T                                                      # (C, HS)
        wall = np.ascontiguousarray(
            np.concatenate([wqk, wv], axis=1)).astype(bf16)            # (C, 192)
        wp = np.ascontiguousarray(Wp[:, h * HS:(h + 1) * HS].T).astype(bf16)  # (HS, C)
        in_maps.append({
            "xt": xt, "wall": wall, "wp": wp, "cc": cc, "ss": ss,
        })
    return in_maps


def run_cores(x, rope, W_attn, W_proj, trace=False):
    """Returns (list of per-core result dicts, BassKernelResults)."""
    nc = _get_nc()
    in_maps = _prep_inputs(x, rope, W_attn, W_proj)
    res = run_bass_kernel_spmd(nc, in_maps, list(range(NCORES)), trace=trace)
    return res


def kernel(x, rope, mask, W_attn, W_proj):
    res = run_cores(x, rope, W_attn, W_proj, trace=False)
    out = np.zeros((T, C), dtype=np.float32)
    for h in range(NCORES):
        r = res.results[h]
        cs = np.asarray(r["cs"], dtype=np.float32).reshape(T, 1)
        out += np.asarray(r["out_u"], dtype=np.float32) / cs
    return out.reshape(B, T, C).astype(np.float32)
